# revision 8
# baseline (speedup 1.0000x reference)
"""Trainium2 Bass kernel for MultiHeadDoublyStochasticSelfAttention.

Problem: b=8, n=1024, f=768, h=12, d=64; 3-step Sinkhorn (eps=1, row/col/row)
on softmax-free exp scores, then attn @ v and output projection.

Sharding: one batch element per NeuronCore (8 cores). Weights replicated.

Single-exp-pass formulation (vs the 2-pass baseline), all in exp domain:
  ST[j,i] = k_j . q_i  (d^-0.5 folded into Wq on host), ET = exp(ST)  [bf16]
  r_i     = sum_j ET[j,i]              (PE ones-matvec over partitions)
  rinv    = 1/r  broadcast to all partitions via a DRAM bounce
  nC_j    = sum_i ET[j,i] * rinv_i     (DVE tensor_tensor_reduce, 4x bf16)
  beta_j  = 1/nC_j                     (per-chunk [128,1] reciprocal)
  av[d,i] = sum_j (beta_j v_jd) ET[j,i]   (PE, lhsT = [beta*v | n*beta])
  out     = av[:64] / av[64]           (row 64 = n*sum ET*beta; 1/r cancels)
Then out^T = Wo @ concat_heads(av-normalized) + bo, host transposes back.
"""

import sys

if "/opt/trn_rl_repo" not in sys.path:
    sys.path.insert(0, "/opt/trn_rl_repo")

from contextlib import ExitStack

import numpy as np

import concourse.bass as bass
import concourse.mybir as mybir
import concourse.tile as tile

B, N, F, H, D = 8, 1024, 768, 12, 64
PC = F // 128        # 6 f-chunks of 128
TC = N // 128        # 8 token chunks of 128
NH = 512             # moving-operand split
LAG = 3              # sweep-2 chunk lag behind sweep-1 (covers rinv DMA RTT)
F32 = mybir.dt.float32
BF16 = mybir.dt.bfloat16
EXP = mybir.ActivationFunctionType.Exp
IDENT = mybir.ActivationFunctionType.Identity
MUL = mybir.AluOpType.mult
ADD = mybir.AluOpType.add
DIV = mybir.AluOpType.divide


def _split_multi_waits(bir_bytes):
    """This container's walrus accepts at most ONE sync wait per instruction
    ("Too many sync wait commands"). Tile's semaphore pass attaches several.
    Rewrite the BIR: spill all but the last wait of each instruction onto
    same-engine NoOps placed directly before it (engines are in-order, so
    semantics are identical)."""
    import json

    d = json.loads(bir_bytes)
    uid = 0
    for fn in d["functions"]:
        for blk in fn["blocks"]:
            out = []
            for ins in blk["instructions"]:
                si = ins.get("sync_info")
                waits = (si or {}).get("on_wait") or []
                if len(waits) > 1:
                    for w in waits[:-1]:
                        uid += 1
                        out.append({
                            "debug": ins.get("debug", 0),
                            "engine": ins["engine"],
                            "ins": [], "outs": [],
                            "name": f"{ins['name']}-w{uid}",
                            "opcode": "NoOp",
                            "sync_info": {"on_update": [], "on_wait": [w]},
                            "text_hint": "split_wait",
                        })
                    si["on_wait"] = [waits[-1]]
                out.append(ins)
            blk["instructions"] = out
    return json.dumps(d).encode()


def build():
    nc = bass.Bass()
    xT = nc.declare_dram_parameter("xT", [F, N], BF16, isOutput=False)
    wqT = nc.declare_dram_parameter("wqT", [F, F], BF16, isOutput=False)
    wkT = nc.declare_dram_parameter("wkT", [F, F], BF16, isOutput=False)
    wvT = nc.declare_dram_parameter("wvT", [F, F], BF16, isOutput=False)
    woT = nc.declare_dram_parameter("woT", [F, F], BF16, isOutput=False)
    bo = nc.declare_dram_parameter("bo", [F], F32, isOutput=False)
    outT = nc.declare_dram_parameter("outT", [F, N], F32, isOutput=True)
    rscratch = nc.dram_tensor("rscratch", [H, N], BF16)
    gscratch = nc.dram_tensor("gscratch", [H, N], F32)

    with tile.TileContext(nc) as tc, ExitStack() as ctx:
        perm = ctx.enter_context(tc.tile_pool(name="perm", bufs=1))
        qt = [perm.tile([128, N], BF16, name=f"qt{i}", tag=f"qt{i}") for i in range(PC)]
        kt = [perm.tile([128, N], BF16, name=f"kt{i}", tag=f"kt{i}") for i in range(PC)]
        # v augmented with a column of n (for the av[64] denom row) per head
        vg = [perm.tile([128, H * (D + 1)], BF16, name=f"vg{i}", tag=f"vg{i}")
              for i in range(TC)]
        ofT = [perm.tile([128, N], BF16, name=f"ofT{i}", tag=f"ofT{i}")
               for i in range(PC)]
        wo_sb = [perm.tile([128, F], BF16, name=f"wo{i}", tag=f"wo{i}")
                 for i in range(PC)]
        bo_sb = perm.tile([128, PC], F32, name="bo_sb", tag="bo_sb")
        ones = perm.tile([128, 1], BF16, name="ones", tag="ones")
        nc.vector.memset(ones, 1.0)
        nc.sync.dma_start(out=bo_sb, in_=bo[:].rearrange("(c p) -> p c", p=128))
        for i in range(PC):
            nc.sync.dma_start(out=wo_sb[i], in_=woT[i * 128:(i + 1) * 128, :])
        for t in range(TC):
            # fill with n; the v-projection copies below overwrite the value
            # columns, leaving each head's 65th column = n
            nc.vector.memset(vg[t], float(N))

        # ---------------- Phase A: q^T, k^T, v projections ----------------
        with tc.tile_pool(name="pxt", bufs=1) as pxt, \
             tc.tile_pool(name="pw", bufs=3 * PC) as pw, \
             tc.tile_pool(name="ppsa", bufs=4, space="PSUM") as ppsa:
            xt = [pxt.tile([128, N], BF16, name=f"xt{i}", tag=f"xt{i}")
                  for i in range(PC)]
            for i in range(PC):
                nc.sync.dma_start(out=xt[i], in_=xT[i * 128:(i + 1) * 128, :])

            for wdram, dst in ((wqT, qt), (wkT, kt)):
                w_sb = []
                for kc in range(PC):
                    w = pw.tile([128, F], BF16, name="w_sb", tag="w")
                    nc.sync.dma_start(out=w, in_=wdram[kc * 128:(kc + 1) * 128, :])
                    w_sb.append(w)
                for mc in range(PC):
                    for hf in range(2):
                        ps = ppsa.tile([128, NH], F32, name="ps_a", tag="psa")
                        for kc in range(PC):
                            nc.tensor.matmul(
                                ps,
                                (w_sb[kc][:, mc * 128:(mc + 1) * 128]),
                                (xt[kc][:, hf * NH:(hf + 1) * NH]),
                                start=(kc == 0), stop=(kc == PC - 1),
                            )
                        nc.vector.tensor_copy(dst[mc][:, hf * NH:(hf + 1) * NH], ps)

            wv_sb = []
            for kc in range(PC):
                w = pw.tile([128, F], BF16, name="wv_sb", tag="w")
                nc.sync.dma_start(out=w, in_=wvT[kc * 128:(kc + 1) * 128, :])
                wv_sb.append(w)
            for t in range(TC):
                for hf, fw in ((0, NH), (1, F - NH)):
                    ps = ppsa.tile([128, NH], F32, name="ps_v", tag="psa")
                    for kc in range(PC):
                        nc.tensor.matmul(
                            ps[:, :fw],
                            (xt[kc][:, t * 128:(t + 1) * 128]),
                            (wv_sb[kc][:, hf * NH:hf * NH + fw]),
                            start=(kc == 0), stop=(kc == PC - 1),
                        )
                    nhd = fw // D
                    src = ps[:, :fw].rearrange("p (h e) -> p h e", e=D)
                    dst3 = vg[t].rearrange("p (h e) -> p h e", e=D + 1)
                    nc.scalar.activation(
                        dst3[:, hf * (NH // D):hf * (NH // D) + nhd, 0:D], src,
                        IDENT,
                    )

        # ---------------- Phase B: per-head sinkhorn attention ----------------
        # 2-stage pipeline at head granularity: sweep-2 (nC/beta/attn@v) of
        # head h-1 interleaves chunk-by-chunk with sweep-1 (ST, exp, row-sum
        # matvec) of head h, keeping PE, ACT and DVE all continuously busy.
        pet = ctx.enter_context(tc.tile_pool(name="pet", bufs=2))
        psml = ctx.enter_context(tc.tile_pool(name="psml", bufs=3))
        pdum = ctx.enter_context(tc.tile_pool(name="pdum", bufs=2))
        prb = ctx.enter_context(tc.tile_pool(name="prb", bufs=2))
        pgb = ctx.enter_context(tc.tile_pool(name="pgb", bufs=2))
        pps_s = ctx.enter_context(tc.tile_pool(name="pps_s", bufs=2, space="PSUM"))
        pps_av = ctx.enter_context(tc.tile_pool(name="pps_av", bufs=2, space="PSUM"))

        def qk(h):
            hc, off = divmod(h, 2)
            off *= D
            return qt[hc][off:off + D, :], kt[hc][off:off + D, :]

        state = {}
        NITER = TC + LAG + 1
        for t in range(H + 1):
            h1 = t if t < H else None       # head in sweep-1
            h2 = t - 1 if t >= 1 else None  # head in sweep-2
            if h1 is not None:
                q1, k1 = qk(h1)
                av1 = pps_av.tile([128, N], F32, name="av_ps", tag="pav")
                et1 = [pet.tile([128, N], BF16, name=f"et{j}", tag=f"et{j}")
                       for j in range(TC)]
            if h2 is not None:
                q2, k2 = qk(h2)
                av2 = state.pop("av")
                et2 = state.pop("et")
                rb2 = state.pop("rb")

            for it in range(NITER):
                # sweep-1: transposed scores + exp
                if h1 is not None and it < TC:
                    for ih in range(2):
                        ps = pps_s.tile([128, NH], F32, name="ps_s", tag="ps")
                        nc.tensor.matmul(
                            ps,
                            k1[:, it * 128:(it + 1) * 128],
                            q1[:, ih * NH:(ih + 1) * NH],
                            start=True, stop=True,
                        )
                        nc.scalar.activation(
                            et1[it][:, ih * NH:(ih + 1) * NH], ps, EXP
                        )
                # sweep-1: row-sum matvec (one chunk behind the exp)
                if h1 is not None and 1 <= it <= TC:
                    jc = it - 1
                    for ih in range(2):
                        nc.tensor.matmul(
                            av1[D:D + 1, ih * NH:(ih + 1) * NH],
                            ones,
                            et1[jc][:, ih * NH:(ih + 1) * NH],
                            start=(jc == 0), stop=(jc == TC - 1),
                            skip_group_check=True,
                        )
                if h1 is not None and it == TC:
                    # rinv = 1/r as a bf16 row, bounced through DRAM into a
                    # broadcast [128, N] tile for the free-dim weighting
                    rrow = psml.tile([1, N], BF16, name="rrow", tag="rrow",
                                     bufs=2)
                    with nc.allow_low_precision(reason="bf16 rinv"):
                        nc.vector.reciprocal(rrow, av1[D:D + 1, :])
                    nc.sync.dma_start(out=rscratch[h1:h1 + 1, :], in_=rrow)
                    rb1 = prb.tile([128, N], BF16, name="rb", tag="rb")
                    rsrc = rscratch[h1:h1 + 1, :]
                    nc.sync.dma_start(
                        out=rb1,
                        in_=bass.AP(tensor=rsrc.tensor, offset=rsrc.offset,
                                    ap=[[0, 128]] + list(rsrc.ap[1:])),
                    )
                    state["av"] = av1
                    state["et"] = et1
                    state["rb"] = rb1

                # sweep-2: nC via weighted free-dim reduce, beta, attn@v
                jc = it - LAG
                if h2 is not None and 0 <= jc < TC:
                    dum = pdum.tile([128, N], BF16, name="dum", tag="dum")
                    ncol = psml.tile([128, 1], F32, name="ncol", tag="ncol")
                    nc.vector.scalar_tensor_tensor(
                        out=dum, in0=et2[jc], scalar=1.0, in1=rb2,
                        op0=MUL, op1=MUL, accum_out=ncol,
                    )
                    bcol = psml.tile([128, 1], F32, name="bcol", tag="bcol")
                    nc.vector.reciprocal(bcol, ncol)
                    bv = psml.tile([128, D + 1], BF16, name="bv", tag="bv")
                    nc.vector.tensor_scalar_mul(
                        bv, vg[jc][:, h2 * (D + 1):(h2 + 1) * (D + 1)], bcol
                    )
                    for ih in range(2):
                        nc.tensor.matmul(
                            av2[0:D + 1, ih * NH:(ih + 1) * NH],
                            bv,
                            et2[jc][:, ih * NH:(ih + 1) * NH],
                            start=(jc == 0), stop=(jc == TC - 1),
                            skip_group_check=True,
                        )
                if h2 is not None and it == LAG + TC:
                    # out_head = av[:64] / av[64]; denominator row broadcast
                    # down the 64 partitions via a DRAM bounce, divide on Pool
                    grow = psml.tile([1, N], F32, name="grow", tag="grow",
                                     bufs=2)
                    nc.vector.reciprocal(grow, av2[D:D + 1, :])
                    nc.sync.dma_start(out=gscratch[h2:h2 + 1, :], in_=grow)
                    gb = pgb.tile([D, N], F32, name="gb", tag="gb")
                    gsrc = gscratch[h2:h2 + 1, :]
                    nc.sync.dma_start(
                        out=gb,
                        in_=bass.AP(tensor=gsrc.tensor, offset=gsrc.offset,
                                    ap=[[0, D]] + list(gsrc.ap[1:])),
                    )
                    hcz, offz = divmod(h2, 2)
                    offz *= D
                    nc.vector.tensor_mul(
                        ofT[hcz][offz:offz + D, :], av2[0:D, :], gb
                    )

        # ---------------- Phase C: output projection + bias ----------------
        for mc in range(PC):
            for hf in range(2):
                ps = pps_s.tile([128, NH], F32, name="ps_o", tag="ps")
                for kc in range(PC):
                    nc.tensor.matmul(
                        ps,
                        (wo_sb[kc][:, mc * 128:(mc + 1) * 128]),
                        (ofT[kc][:, hf * NH:(hf + 1) * NH]),
                        start=(kc == 0), stop=(kc == PC - 1),
                    )
                o_sb = pdum.tile([128, NH], F32, name="o_sb", tag="osb", bufs=2)
                nc.scalar.activation(o_sb, ps, IDENT, bias=bo_sb[:, mc:mc + 1])
                nc.sync.dma_start(
                    out=outT[mc * 128:(mc + 1) * 128, hf * NH:(hf + 1) * NH],
                    in_=o_sb,
                )

    orig_to_json = nc.to_json_bytes
    nc.to_json_bytes = lambda: _split_multi_waits(orig_to_json())
    return nc


_NC = None


def _get_nc():
    global _NC
    if _NC is None:
        _NC = build()
    return _NC


def make_in_maps(x, Wq, Wk, Wv, Wo, bo):
    import ml_dtypes

    bf16 = ml_dtypes.bfloat16
    scale = np.float32(D ** -0.5)
    wq_t = np.ascontiguousarray((Wq.astype(np.float32) * scale).T.astype(bf16))
    wk_t = np.ascontiguousarray(Wk.T.astype(bf16))
    wv_t = np.ascontiguousarray(Wv.T.astype(bf16))
    wo_t = np.ascontiguousarray(Wo.T.astype(bf16))
    bo_c = np.ascontiguousarray(bo.astype(np.float32))
    maps = []
    for c in range(B):
        maps.append({
            "xT": np.ascontiguousarray(x[c].T.astype(bf16)),
            "wqT": wq_t, "wkT": wk_t, "wvT": wv_t, "woT": wo_t, "bo": bo_c,
        })
    return maps


def kernel(x, Wq, Wk, Wv, Wo, bo):
    from concourse.bass_utils import run_bass_kernel_spmd

    x = np.asarray(x)
    nc = _get_nc()
    in_maps = make_in_maps(np.asarray(x), np.asarray(Wq), np.asarray(Wk),
                           np.asarray(Wv), np.asarray(Wo), np.asarray(bo))
    res = run_bass_kernel_spmd(nc, in_maps, core_ids=list(range(B)))
    out = np.stack([res.results[c]["outT"].T for c in range(B)], axis=0)
    return out.astype(np.float32)
